# revision 11
# baseline (speedup 1.0000x reference)
"""Exact attention (B=2, N=2048, H=16, D=64, fp32) on 8 Trainium2 NeuronCores.

Sharding: the 32 (batch, head) pairs are split across 8 cores, 4 heads per
core. Each core computes full (non-causal, unscaled) attention for its heads.

v5 design (HW-measured: ACT exp roofline ~948ns per [128,1024] instruction
= 121us/core; cross-engine semaphore handoffs cost 100s of ns each, so the
schedule is built to make every wait PRE-SATISFIED):
  - Q/K host-pre-transposed to [h, d, n]: direct DMA into [d, n] SBUF
    layout (2KB-contiguous runs), no on-device staging transposes.
  - SINGLE-HEAD steps: each step g = (pair, nh, head, mb) does one
    [128, 1024] S^T chunk: 2 QK matmuls -> 1 exp -> (2 steps later) 2 AV
    matmuls. With one exp per step, the 2-deep S^T PSUM ring spans TWO
    steps (~2.1us), so the exp that frees a buffer completes long before
    the next QK needs it - the in-order PE queue never block-waits on ACT,
    and AV (emitted with a 2-step lag, pt ring 6 deep) never does either.
  - O^T[65, n] += V'^T P^T with V' = [V | ones]; row 64 accumulates the
    softmax denominators. opool bufs=2 so the finalize of head h overlaps
    head h+1's accumulation.
  - Finalize has NO PE work and no transposes: DVE reciprocal of the
    denominator row, gpsimd partition_broadcast, DVE multiply straight out
    of PSUM, output in [h, d, n] (host transposes back to [h, n, d]).

Numerics: matmuls use float32r (fp32 with 11-bit mantissa, full PE rate).
"""

import os
import sys

os.environ.setdefault("MYCRO_LOCAL_CACHE", "1")
sys.path.insert(0, "/opt/trn_rl_repo")

import ml_dtypes
import numpy as np

import concourse.bacc as bacc
import concourse.mybir as mybir
import concourse.tile as tile
from concourse.bass_utils import run_bass_kernel_spmd

f32 = mybir.dt.float32
f32r = mybir.dt.float32r
bf16 = mybir.dt.bfloat16

B, N, H, D = 2, 2048, 16, 64
HEADS_PER_CORE = 4
N_CORES = 8
NH = 1024          # n-half width
N_MB = N // 128    # 16 m-blocks of 128 rows
DV = D + 1         # V plus ones column
AV_LAG = 4         # steps between exp and its consuming AV matmuls


def emit_body(nc, qT, kT, v, outT, pools):
    """One full attention pass for 4 heads. qT/kT [4, D, N], v [4, N, D],
    outT [4, D, N] (host un-transposes)."""
    const, qkt, vt_p, spool, ppool, opool, finsb = pools

    def emit_inputs(pair):
        h0, h1 = 2 * pair, 2 * pair + 1
        qt = qkt.tile([128, N], f32r, name=f"qt_{pair}", tag="qt")
        kt = qkt.tile([128, N], f32r, name=f"kt_{pair}", tag="kt")
        for c in range(4):
            csl = slice(c * 512, (c + 1) * 512)
            for dst, src in ((qt, qT), (kt, kT)):
                for hh, plo in ((h0, 0), (h1, 64)):
                    nc.sync.dma_start(
                        out=dst[plo:plo + 64, csl],
                        in_=src.bitcast(f32r)[hh, :, csl])
        vts = []
        for i, hh in enumerate((h0, h1)):
            vt = vt_p.tile([128, N_MB, DV], bf16, name=f"vt_{hh}", tag=f"vt{i}")
            nc.sync.dma_start(out=vt, in_=v[hh])
            vts.append(vt)
        return qt, kt, vts

    state = [emit_inputs(0), emit_inputs(1)]
    oaccs = {}

    def emit_qk(pair, nh, i, mb):
        qt, kt, _ = state[pair]
        msl = slice(mb * 128, (mb + 1) * 128)
        plo = 64 * i
        sp = spool.tile([128, NH], f32,
                        name=f"sp_{pair}_{nh}_{i}_{mb}", tag="s")
        for j in range(NH // 512):
            jsl = slice(nh * NH + j * 512, nh * NH + (j + 1) * 512)
            nc.tensor.matmul(
                out=sp[:, j * 512:(j + 1) * 512], lhsT=kt[plo:plo + 64, msl],
                rhs=qt[plo:plo + 64, jsl], start=True, stop=True)
        pt = ppool.tile([128, NH], bf16,
                        name=f"pt_{pair}_{nh}_{i}_{mb}", tag="p")
        nc.scalar.activation(
            out=pt, in_=sp, func=mybir.ActivationFunctionType.Exp)
        return pt

    def emit_av(pair, nh, i, mb, pt):
        _, _, vts = state[pair]
        if mb == 0:
            oaccs[(pair, nh, i)] = opool.tile(
                [65, NH], f32, name=f"o_{pair}_{nh}_{i}", tag="o")
        oo = oaccs[(pair, nh, i)]
        for j in range(NH // 512):
            osl = slice(j * 512, (j + 1) * 512)
            nc.tensor.matmul(
                out=oo[:, osl], lhsT=vts[i][:, mb, :], rhs=pt[:, osl],
                start=mb == 0, stop=mb == N_MB - 1)
        if mb == N_MB - 1:
            # finalize this head's n-half: normalize O^T rows 0:63 by the
            # reciprocal of denominator row 64, all in [d, n] layout
            hh = 2 * pair + i
            rcp = finsb.tile([1, NH], f32, name=f"rcp_{pair}_{nh}_{i}", tag="rcp")
            nc.vector.reciprocal(rcp, oo[64:65, :])
            bc = finsb.tile([64, NH], f32, name=f"bc_{pair}_{nh}_{i}", tag="bc")
            nc.gpsimd.partition_broadcast(bc, rcp)
            onorm = finsb.tile([64, NH], f32, name=f"on_{pair}_{nh}_{i}",
                               tag="onorm")
            nc.vector.tensor_mul(onorm, oo[0:64, :], bc)
            nc.sync.dma_start(
                out=outT[hh, :, nh * NH:(nh + 1) * NH], in_=onorm)

    # Single-head software pipeline with a 2-step AV lag.
    steps = [(p, n, i, m) for p in range(2) for n in range(2)
             for i in range(2) for m in range(N_MB)]
    pending = []
    for g, (pair, nh, i, mb) in enumerate(steps):
        pt = emit_qk(pair, nh, i, mb)
        pending.append((pair, nh, i, mb, pt))
        if len(pending) > AV_LAG:
            emit_av(*pending.pop(0))
    while pending:
        emit_av(*pending.pop(0))


def build(repeat=1):
    nc = bacc.Bacc("TRN2", target_bir_lowering=False, debug=False)
    qT = nc.dram_tensor("qT", [HEADS_PER_CORE, D, N], f32, kind="ExternalInput").ap()
    kT = nc.dram_tensor("kT", [HEADS_PER_CORE, D, N], f32, kind="ExternalInput").ap()
    v = nc.dram_tensor("v", [HEADS_PER_CORE, 128, N_MB, DV], bf16,
                       kind="ExternalInput").ap()
    outT = nc.dram_tensor("outT", [HEADS_PER_CORE, D, N], f32,
                          kind="ExternalOutput").ap()

    from contextlib import ExitStack
    with tile.TileContext(nc) as tc, ExitStack() as ctx:
        qkt = ctx.enter_context(tc.tile_pool(name="qkt", bufs=2))
        vt_p = ctx.enter_context(tc.tile_pool(name="vt", bufs=2))
        spool = ctx.enter_context(tc.tile_pool(name="spool", bufs=2, space="PSUM"))
        ppool = ctx.enter_context(tc.tile_pool(name="ppool", bufs=8))
        opool = ctx.enter_context(tc.tile_pool(name="opool", bufs=2, space="PSUM"))
        finsb = ctx.enter_context(tc.tile_pool(name="finsb", bufs=2))

        pools = ({}, qkt, vt_p, spool, ppool, opool, finsb)

        if repeat == 1:
            emit_body(nc, qT, kT, v, outT, pools)
        else:
            # 2x-unrolled timing loop: For_i puts an all-engine barrier in
            # its per-iteration semaphore-reset block, so fewer, bigger
            # iterations amortize the pipeline drain/refill.
            un = 2 if repeat >= 2 else 1
            with tc.For_i(0, repeat // un, 1, hint_engines=(
                    mybir.EngineType.PE, mybir.EngineType.Activation,
                    mybir.EngineType.DVE, mybir.EngineType.SP,
                    mybir.EngineType.Pool)):
                for _ in range(un):
                    emit_body(nc, qT, kT, v, outT, pools)
            for _ in range(repeat - (repeat // un) * un):
                emit_body(nc, qT, kT, v, outT, pools)

    nc.compile()
    return nc


_NC_CACHE = {}


def _get_nc(repeat=1):
    if repeat not in _NC_CACHE:
        _NC_CACHE[repeat] = build(repeat)
    return _NC_CACHE[repeat]


def run_sharded(query, key, value, repeat=1, **spmd_kwargs):
    """query/key/value: [B, N, H, D] fp32 -> out [B, H, N, D] fp32."""
    nc = _get_nc(repeat)
    # [B, N, H, D] -> [B*H, N, D]; q/k additionally -> [B*H, D, N]
    qh = np.ascontiguousarray(np.transpose(query, (0, 2, 3, 1))).reshape(B * H, D, N)
    kh = np.ascontiguousarray(np.transpose(key, (0, 2, 3, 1))).reshape(B * H, D, N)
    vh = np.ascontiguousarray(np.transpose(value, (0, 2, 1, 3))).reshape(
        B * H, N, D).astype(ml_dtypes.bfloat16)
    vh = np.concatenate(
        [vh, np.ones((B * H, N, 1), dtype=ml_dtypes.bfloat16)], axis=2)
    # [BH, N=(mb p), 65] -> [BH, p, mb, 65] so each SBUF partition's data is
    # one contiguous 2080B run
    vh = np.ascontiguousarray(
        vh.reshape(B * H, N_MB, 128, DV).transpose(0, 2, 1, 3))
    in_maps = [
        {
            "qT": qh[c * HEADS_PER_CORE:(c + 1) * HEADS_PER_CORE],
            "kT": kh[c * HEADS_PER_CORE:(c + 1) * HEADS_PER_CORE],
            "v": vh[c * HEADS_PER_CORE:(c + 1) * HEADS_PER_CORE],
        }
        for c in range(N_CORES)
    ]
    res = run_bass_kernel_spmd(nc, in_maps, core_ids=list(range(N_CORES)),
                               **spmd_kwargs)
    outs = np.stack([res.results[c]["outT"] for c in range(N_CORES)])  # [8,4,D,N]
    return np.ascontiguousarray(
        outs.reshape(B, H, D, N).transpose(0, 1, 3, 2))


def kernel(query, key, value):
    query = np.asarray(query, dtype=np.float32)
    key = np.asarray(key, dtype=np.float32)
    value = np.asarray(value, dtype=np.float32)
    return run_sharded(query, key, value)


if __name__ == "__main__":
    rng = np.random.default_rng(0)
    q = rng.standard_normal((B, N, H, D), dtype=np.float32)
    k = rng.standard_normal((B, N, H, D), dtype=np.float32)
    v = rng.standard_normal((B, N, H, D), dtype=np.float32)
    o = kernel(q, k, v)
    print("out shape:", o.shape, o.dtype)


# revision 12
# speedup vs baseline: 1.0970x; 1.0970x over previous
"""Exact attention (B=2, N=2048, H=16, D=64, fp32) on 8 Trainium2 NeuronCores.

Sharding: the 32 (batch, head) pairs are split across 8 cores, 4 heads per
core. Each core computes full (non-causal, unscaled) attention for its heads.

v5 design (HW-measured: ACT exp roofline ~948ns per [128,1024] instruction
= 121us/core; cross-engine semaphore handoffs cost 100s of ns each, so the
schedule is built to make every wait PRE-SATISFIED):
  - Q/K host-pre-transposed to [h, d, n]: direct DMA into [d, n] SBUF
    layout (2KB-contiguous runs), no on-device staging transposes.
  - SINGLE-HEAD steps: each step g = (pair, nh, head, mb) does one
    [128, 1024] S^T chunk: 2 QK matmuls -> 1 exp -> (2 steps later) 2 AV
    matmuls. With one exp per step, the 2-deep S^T PSUM ring spans TWO
    steps (~2.1us), so the exp that frees a buffer completes long before
    the next QK needs it - the in-order PE queue never block-waits on ACT,
    and AV (emitted with a 2-step lag, pt ring 6 deep) never does either.
  - O^T[65, n] += V'^T P^T with V' = [V | ones]; row 64 accumulates the
    softmax denominators. opool bufs=2 so the finalize of head h overlaps
    head h+1's accumulation.
  - Finalize has NO PE work and no transposes: DVE reciprocal of the
    denominator row, gpsimd partition_broadcast, DVE multiply straight out
    of PSUM, output in [h, d, n] (host transposes back to [h, n, d]).

Numerics: matmuls use float32r (fp32 with 11-bit mantissa, full PE rate).
"""

import os
import sys

os.environ.setdefault("MYCRO_LOCAL_CACHE", "1")
sys.path.insert(0, "/opt/trn_rl_repo")

import ml_dtypes
import numpy as np

import concourse.bacc as bacc
import concourse.mybir as mybir
import concourse.tile as tile
from concourse.bass_utils import run_bass_kernel_spmd

f32 = mybir.dt.float32
f32r = mybir.dt.float32r
bf16 = mybir.dt.bfloat16

B, N, H, D = 2, 2048, 16, 64
HEADS_PER_CORE = 4
N_CORES = 8
NH = 1024          # n-half width
N_MB = N // 128    # 16 m-blocks of 128 rows
DV = D + 1         # V plus ones column
AV_LAG = 3         # steps between exp and its consuming AV matmuls


def emit_body(nc, qT, kT, v, outT, pools):
    """One full attention pass for 4 heads. qT/kT [4, D, N], v [4, N, D],
    outT [4, D, N] (host un-transposes)."""
    const, qkt, vt_p, spool, ppool, opool, finsb = pools

    def emit_inputs(pair):
        h0, h1 = 2 * pair, 2 * pair + 1
        qt = qkt.tile([128, N], f32r, name=f"qt_{pair}", tag="qt")
        kt = qkt.tile([128, N], f32r, name=f"kt_{pair}", tag="kt")
        for c in range(4):
            csl = slice(c * 512, (c + 1) * 512)
            for dst, src in ((qt, qT), (kt, kT)):
                for hh, plo in ((h0, 0), (h1, 64)):
                    nc.sync.dma_start(
                        out=dst[plo:plo + 64, csl],
                        in_=src.bitcast(f32r)[hh, :, csl])
        vts = []
        for i, hh in enumerate((h0, h1)):
            vt = vt_p.tile([128, N_MB, DV], bf16, name=f"vt_{hh}", tag=f"vt{i}")
            nc.sync.dma_start(
                out=vt[:, :, 0:64],
                in_=v[hh].rearrange("(mb p) d -> p mb d", p=128),
            )
            nc.vector.tensor_copy(vt[:, :, 64:65], const["ones"])
            vts.append(vt)
        return qt, kt, vts

    state = [emit_inputs(0), emit_inputs(1)]
    oaccs = {}

    def emit_qk(pair, nh, i, mb):
        qt, kt, _ = state[pair]
        msl = slice(mb * 128, (mb + 1) * 128)
        plo = 64 * i
        sp = spool.tile([128, NH], f32,
                        name=f"sp_{pair}_{nh}_{i}_{mb}", tag="s")
        for j in range(NH // 512):
            jsl = slice(nh * NH + j * 512, nh * NH + (j + 1) * 512)
            nc.tensor.matmul(
                out=sp[:, j * 512:(j + 1) * 512], lhsT=kt[plo:plo + 64, msl],
                rhs=qt[plo:plo + 64, jsl], start=True, stop=True)
        pt = ppool.tile([128, NH], bf16,
                        name=f"pt_{pair}_{nh}_{i}_{mb}", tag="p")
        nc.scalar.activation(
            out=pt, in_=sp, func=mybir.ActivationFunctionType.Exp)
        return pt

    def emit_av(pair, nh, i, mb, pt):
        _, _, vts = state[pair]
        if mb == 0:
            oaccs[(pair, nh, i)] = opool.tile(
                [65, NH], f32, name=f"o_{pair}_{nh}_{i}", tag="o")
        oo = oaccs[(pair, nh, i)]
        for j in range(NH // 512):
            osl = slice(j * 512, (j + 1) * 512)
            nc.tensor.matmul(
                out=oo[:, osl], lhsT=vts[i][:, mb, :], rhs=pt[:, osl],
                start=mb == 0, stop=mb == N_MB - 1)
        if mb == N_MB - 1:
            # finalize this head's n-half: normalize O^T rows 0:63 by the
            # reciprocal of denominator row 64, all in [d, n] layout
            hh = 2 * pair + i
            rcp = finsb.tile([1, NH], f32, name=f"rcp_{pair}_{nh}_{i}", tag="rcp")
            nc.vector.reciprocal(rcp, oo[64:65, :])
            bc = finsb.tile([64, NH], f32, name=f"bc_{pair}_{nh}_{i}", tag="bc")
            nc.gpsimd.partition_broadcast(bc, rcp)
            onorm = finsb.tile([64, NH], f32, name=f"on_{pair}_{nh}_{i}",
                               tag="onorm")
            nc.vector.tensor_mul(onorm, oo[0:64, :], bc)
            nc.sync.dma_start(
                out=outT[hh, :, nh * NH:(nh + 1) * NH], in_=onorm)

    # Single-head software pipeline with a 2-step AV lag.
    steps = [(p, n, i, m) for p in range(2) for n in range(2)
             for i in range(2) for m in range(N_MB)]
    pending = []
    for g, (pair, nh, i, mb) in enumerate(steps):
        pt = emit_qk(pair, nh, i, mb)
        pending.append((pair, nh, i, mb, pt))
        if len(pending) > AV_LAG:
            emit_av(*pending.pop(0))
    while pending:
        emit_av(*pending.pop(0))


def build(repeat=1):
    nc = bacc.Bacc("TRN2", target_bir_lowering=False, debug=False)
    qT = nc.dram_tensor("qT", [HEADS_PER_CORE, D, N], f32, kind="ExternalInput").ap()
    kT = nc.dram_tensor("kT", [HEADS_PER_CORE, D, N], f32, kind="ExternalInput").ap()
    v = nc.dram_tensor("v", [HEADS_PER_CORE, N, D], bf16, kind="ExternalInput").ap()
    outT = nc.dram_tensor("outT", [HEADS_PER_CORE, D, N], f32,
                          kind="ExternalOutput").ap()

    from contextlib import ExitStack
    with tile.TileContext(nc) as tc, ExitStack() as ctx:
        const_pool = ctx.enter_context(tc.tile_pool(name="const", bufs=1))
        ones = const_pool.tile([128, N_MB, 1], bf16, name="ones")
        nc.vector.memset(ones, 1.0)

        qkt = ctx.enter_context(tc.tile_pool(name="qkt", bufs=2))
        vt_p = ctx.enter_context(tc.tile_pool(name="vt", bufs=2))
        spool = ctx.enter_context(tc.tile_pool(name="spool", bufs=2, space="PSUM"))
        ppool = ctx.enter_context(tc.tile_pool(name="ppool", bufs=6))
        opool = ctx.enter_context(tc.tile_pool(name="opool", bufs=2, space="PSUM"))
        finsb = ctx.enter_context(tc.tile_pool(name="finsb", bufs=2))

        pools = ({"ones": ones}, qkt, vt_p, spool, ppool, opool, finsb)

        if repeat == 1:
            emit_body(nc, qT, kT, v, outT, pools)
        else:
            # 2x-unrolled timing loop: For_i puts an all-engine barrier in
            # its per-iteration semaphore-reset block, so fewer, bigger
            # iterations amortize the pipeline drain/refill.
            un = 2 if repeat >= 2 else 1
            with tc.For_i(0, repeat // un, 1, hint_engines=(
                    mybir.EngineType.PE, mybir.EngineType.Activation,
                    mybir.EngineType.DVE, mybir.EngineType.SP,
                    mybir.EngineType.Pool)):
                for _ in range(un):
                    emit_body(nc, qT, kT, v, outT, pools)
            for _ in range(repeat - (repeat // un) * un):
                emit_body(nc, qT, kT, v, outT, pools)

    nc.compile()
    return nc


_NC_CACHE = {}


def _get_nc(repeat=1):
    if repeat not in _NC_CACHE:
        _NC_CACHE[repeat] = build(repeat)
    return _NC_CACHE[repeat]


def run_sharded(query, key, value, repeat=1, **spmd_kwargs):
    """query/key/value: [B, N, H, D] fp32 -> out [B, H, N, D] fp32."""
    nc = _get_nc(repeat)
    # [B, N, H, D] -> [B*H, N, D]; q/k additionally -> [B*H, D, N]
    qh = np.ascontiguousarray(np.transpose(query, (0, 2, 3, 1))).reshape(B * H, D, N)
    kh = np.ascontiguousarray(np.transpose(key, (0, 2, 3, 1))).reshape(B * H, D, N)
    vh = np.ascontiguousarray(np.transpose(value, (0, 2, 1, 3))).reshape(
        B * H, N, D).astype(ml_dtypes.bfloat16)
    in_maps = [
        {
            "qT": qh[c * HEADS_PER_CORE:(c + 1) * HEADS_PER_CORE],
            "kT": kh[c * HEADS_PER_CORE:(c + 1) * HEADS_PER_CORE],
            "v": vh[c * HEADS_PER_CORE:(c + 1) * HEADS_PER_CORE],
        }
        for c in range(N_CORES)
    ]
    res = run_bass_kernel_spmd(nc, in_maps, core_ids=list(range(N_CORES)),
                               **spmd_kwargs)
    outs = np.stack([res.results[c]["outT"] for c in range(N_CORES)])  # [8,4,D,N]
    return np.ascontiguousarray(
        outs.reshape(B, H, D, N).transpose(0, 1, 3, 2))


def kernel(query, key, value):
    query = np.asarray(query, dtype=np.float32)
    key = np.asarray(key, dtype=np.float32)
    value = np.asarray(value, dtype=np.float32)
    return run_sharded(query, key, value)


if __name__ == "__main__":
    rng = np.random.default_rng(0)
    q = rng.standard_normal((B, N, H, D), dtype=np.float32)
    k = rng.standard_normal((B, N, H, D), dtype=np.float32)
    v = rng.standard_normal((B, N, H, D), dtype=np.float32)
    o = kernel(q, k, v)
    print("out shape:", o.shape, o.dtype)
